# revision 1
# baseline (speedup 1.0000x reference)
"""Bahdanau-style attention kernel for Trainium2, data-parallel over batch
across 8 NeuronCores.

Reference computation (per batch b):
    e_proj = enc[b] @ We.T            # [S, D]   (We = W[:, 512:], [D, E])
    energy = tanh(e_proj + hidden[b] @ Wh.T + bias)
    scores = energy @ v               # [S]
    attn   = softmax(scores)          # [1, S]

Shapes: B=32, S=2048, E=1024, D=512.  Each core handles 4 batches.

Device-side design (per core):
  - enc is DMA-loaded f32 (HWDGE fast path) and cast to fp16 on VectorE,
    then transposed on TensorE (identity matmul) so the contraction dim e
    lands on partitions.  fp16 keeps 10 mantissa bits (end-to-end attn
    error ~1.2e-3) while streaming the PE at full rate with fast weight
    loads.
  - main matmul: psum[d128, s512] += WeT[e128, d128].T @ encT[e128, s512]
  - tanh fused with the (h_proj + b) bias via ScalarE activation
    (per-partition bias, since d is the partition dim).
  - scores via TensorE matvec with v; softmax single-partition on
    VectorE/ScalarE only (keeps the PE stream free of stalls).
"""

import numpy as np

B, S, E, D = 32, 2048, 1024, 512
N_CORES = 8
BP = B // N_CORES  # batches per core = 4
SBLK = 512  # s-block (psum free dim)
N_SBLK = S // SBLK  # 4
N_ST = SBLK // 128  # 4 s-subtiles per block
N_EC = E // 128  # 8 e-chunks
N_DP = D // 128  # 4 d-chunks
N_KC = D // 128  # 4 k-chunks (hidden proj contraction)

_CACHE = {}


def _build(debug_dumps=False):
    from contextlib import ExitStack

    import concourse.bass as bass
    import concourse.tile as tile
    from concourse import bacc, mybir
    from concourse.masks import make_identity

    F32 = mybir.dt.float32
    F16 = mybir.dt.float16
    AF = mybir.ActivationFunctionType
    ALU = mybir.AluOpType
    AX = mybir.AxisListType

    nc = bacc.Bacc("TRN2", target_bir_lowering=False, debug=False,
                   num_devices=N_CORES)

    hid_d = nc.dram_tensor("hidden", [BP, D], F32, kind="ExternalInput").ap()
    enc_d = nc.dram_tensor("enc", [BP, S, E], F32, kind="ExternalInput").ap()
    w_d = nc.dram_tensor("W", [D, D + E], F32, kind="ExternalInput").ap()
    b_d = nc.dram_tensor("b", [D], F32, kind="ExternalInput").ap()
    v_d = nc.dram_tensor("v", [D], F32, kind="ExternalInput").ap()
    out_d = nc.dram_tensor("out", [BP, S], F32, kind="ExternalOutput").ap()
    if debug_dumps:
        dbg_scores = nc.dram_tensor(
            "dbg_scores", [BP, S], F32, kind="ExternalOutput").ap()
        dbg_energy = nc.dram_tensor(
            "dbg_energy", [128, N_DP, SBLK], F16, kind="ExternalOutput").ap()
        dbg_enct = nc.dram_tensor(
            "dbg_enct", [128, N_EC, S], F16, kind="ExternalOutput").ap()
        dbg_hbt = nc.dram_tensor(
            "dbg_hbt", [128, N_DP, BP], F32, kind="ExternalOutput").ap()
        dbg_wet = nc.dram_tensor(
            "dbg_wet", [128, N_EC, D], F16, kind="ExternalOutput").ap()

    with tile.TileContext(nc) as tc, ExitStack() as ctx:
        consts = ctx.enter_context(tc.tile_pool(name="consts", bufs=1))
        enc_pool = ctx.enter_context(tc.tile_pool(name="enc", bufs=2))
        enc16_pool = ctx.enter_context(tc.tile_pool(name="enc16", bufs=3))
        work = ctx.enter_context(tc.tile_pool(name="work", bufs=2))
        small = ctx.enter_context(tc.tile_pool(name="small", bufs=2))
        sm1 = ctx.enter_context(tc.tile_pool(name="sm1", bufs=1))
        ps = ctx.enter_context(tc.tile_pool(name="ps", bufs=3, space="PSUM"))
        ps2 = ctx.enter_context(tc.tile_pool(name="ps2", bufs=2, space="PSUM"))

        identity = consts.tile([128, 128], F32)
        make_identity(nc, identity)
        identity16 = consts.tile([128, 128], F16)
        make_identity(nc, identity16)

        # ---- load weights & small inputs ----
        # w_sb shares the encT tag slots (setup-only lifetime).
        w_sb = work.tile([128, N_DP, D + E], F32, tag="encT")
        nc.sync.dma_start(out=w_sb, in_=w_d.rearrange("(dp p) q -> p dp q", p=128))
        hid_sb = consts.tile([BP, D], F32)
        nc.sync.dma_start(out=hid_sb, in_=hid_d)
        b_sb4 = consts.tile([N_DP, 128], F32)
        nc.sync.dma_start(out=b_sb4, in_=b_d.rearrange("(dp q) -> dp q", q=128))
        v_sb4 = consts.tile([N_DP, 128], F32)
        nc.sync.dma_start(out=v_sb4, in_=v_d.rearrange("(dp q) -> dp q", q=128))

        # preload the exp/tanh activation table early (overlaps with DMAs)
        warm = consts.tile([1, 1], F32)
        nc.vector.memset(warm, 0.0)
        nc.scalar.activation(warm, warm, AF.Tanh)

        # ---- transpose We -> WeT [e, d] (fp16), Wh -> WhT [k, d] ----
        wet_sb = consts.tile([128, N_EC, D], F16)
        for ec in range(N_EC):
            pt = ps.tile([128, 512], F32, tag="ptr")
            with tc.tile_critical():
                for dp in range(N_DP):
                    nc.tensor.matmul(
                        pt[:, dp * 128:(dp + 1) * 128],
                        w_sb[:, dp, D + ec * 128: D + (ec + 1) * 128],
                        identity, is_transpose=True,
                        start=(dp == 0), stop=(dp == N_DP - 1),
                    )
            nc.vector.tensor_copy(wet_sb[:, ec, :], pt)

        wht_sb = consts.tile([128, N_KC, D], F32)
        for kc in range(N_KC):
            pt = ps.tile([128, 512], F32, tag="ptr")
            with tc.tile_critical():
                for dp in range(N_DP):
                    nc.tensor.matmul(
                        pt[:, dp * 128:(dp + 1) * 128],
                        w_sb[:, dp, kc * 128:(kc + 1) * 128],
                        identity, is_transpose=True,
                        start=(dp == 0), stop=(dp == N_DP - 1),
                    )
            nc.scalar.copy(wht_sb[:, kc, :], pt)

        # ---- hidden^T [k, b] ----
        hidt_sb = consts.tile([128, N_KC, BP], F32)
        for kc in range(N_KC):
            pt = ps2.tile([128, 16], F32, tag="sc")
            nc.tensor.transpose(
                pt[:, 0:BP], hid_sb[:, kc * 128:(kc + 1) * 128],
                identity[0:BP, 0:BP],
            )
            nc.vector.tensor_copy(hidt_sb[:, kc, :], pt[:, 0:BP])

        # ---- b^T, v^T  [128, dp] ----
        bt_sb = consts.tile([128, N_DP], F32)
        pt = ps2.tile([128, 16], F32, tag="sc")
        nc.tensor.transpose(pt[:, 0:N_DP], b_sb4, identity[0:N_DP, 0:N_DP])
        nc.vector.tensor_copy(bt_sb, pt[:, 0:N_DP])

        vt_sb = consts.tile([128, N_DP], F16)
        pt = ps2.tile([128, 16], F32, tag="sc")
        nc.tensor.transpose(pt[:, 0:N_DP], v_sb4, identity[0:N_DP, 0:N_DP])
        nc.vector.tensor_copy(vt_sb, pt[:, 0:N_DP])

        # ---- h_projT + bias -> hbT [128, dp, b] ----
        hbt_sb = consts.tile([128, N_DP, BP], F32)
        for dp in range(N_DP):
            ph = ps2.tile([128, 16], F32, tag="sc")
            for kc in range(N_KC):
                nc.tensor.matmul(
                    ph[:, 0:BP],
                    wht_sb[:, kc, dp * 128:(dp + 1) * 128],
                    hidt_sb[:, kc, :],
                    start=(kc == 0), stop=(kc == N_KC - 1),
                )
            nc.vector.tensor_scalar_add(
                hbt_sb[:, dp, :], ph[:, 0:BP], bt_sb[:, dp:dp + 1]
            )

        # ---- main loop ----
        for bi in range(BP):
            # HWDGE f32 load (fast path), then cast f32 -> fp16 on VectorE
            # (SWDGE cast-DMA cannot keep the PE fed).
            enc_nat = [None] * N_SBLK
            for sblk in range(N_SBLK):
                enc32 = enc_pool.tile([128, N_ST, E], F32, tag="enc32")
                nc.sync.dma_start(
                    out=enc32,
                    in_=enc_d[bi, sblk * SBLK:(sblk + 1) * SBLK, :].rearrange(
                        "(st p) e -> p st e", p=128
                    ),
                )
                enc_tile = enc16_pool.tile([128, N_ST, E], F16, tag="enc_nat")
                enc_nat[sblk] = enc_tile
                nc.vector.tensor_copy(enc_tile[:, 0:2, :], enc32[:, 0:2, :])
                nc.vector.tensor_copy(enc_tile[:, 2:4, :], enc32[:, 2:4, :])

            # transpose the whole batch: encT[e, s] for s in [0, 2048)
            enct_sb = work.tile([128, N_EC, S], F16, tag="encT")
            for sblk in range(N_SBLK):
                for ech in range(N_EC // 2):
                    pt = ps.tile([128, 1024], F16, tag="ptr")
                    with tc.tile_critical():
                        for half in range(2):
                            ec = ech * 2 + half
                            for st in range(N_ST):
                                nc.tensor.matmul(
                                    pt[:, half * 512 + st * 128:
                                       half * 512 + (st + 1) * 128],
                                    enc_nat[sblk][:, st, ec * 128:(ec + 1) * 128],
                                    identity16, is_transpose=True,
                                    start=(half == 0 and st == 0),
                                    stop=(half == 1 and st == N_ST - 1),
                                )
                    dst = enct_sb[:, ech * 2:ech * 2 + 2,
                                  sblk * SBLK:(sblk + 1) * SBLK]
                    src_ap = pt.rearrange("p (h f) -> p h f", h=2)
                    if (sblk * (N_EC // 2) + ech) % 2 == 0:
                        nc.vector.tensor_copy(dst, src_ap)
                    else:
                        nc.scalar.copy(dst, src_ap)

            scores_sb = small.tile([1, S], F32, tag="scores")
            for sblk in range(N_SBLK):
                energy_sb = work.tile([128, N_DP, SBLK], F16, tag="energy")
                for dp in range(N_DP):
                    pe = ps.tile([128, SBLK], F32, tag="pe")
                    for ec in range(N_EC):
                        nc.tensor.matmul(
                            pe,
                            wet_sb[:, ec, dp * 128:(dp + 1) * 128],
                            enct_sb[:, ec, sblk * SBLK:(sblk + 1) * SBLK],
                            start=(ec == 0), stop=(ec == N_EC - 1),
                        )
                    nc.scalar.activation(
                        energy_sb[:, dp, :], pe, AF.Tanh,
                        bias=hbt_sb[:, dp, bi:bi + 1], scale=1.0,
                    )

                psc = ps2.tile([1, SBLK], F32, tag="sc")
                for dp in range(N_DP):
                    nc.tensor.matmul(
                        psc, vt_sb[:, dp:dp + 1], energy_sb[:, dp, :],
                        start=(dp == 0), stop=(dp == N_DP - 1),
                    )
                nc.scalar.copy(scores_sb[:, sblk * SBLK:(sblk + 1) * SBLK], psc)

                if debug_dumps and bi == 0 and sblk == 0:
                    nc.sync.dma_start(out=dbg_energy, in_=energy_sb)

            if debug_dumps and bi == 0:
                nc.sync.dma_start(out=dbg_enct, in_=enct_sb)

            # ---- softmax over S=2048 for this batch ----
            # Single-partition softmax on VectorE/ScalarE only: keeps the
            # PE stream free of cross-batch stalls (no transposes, no
            # broadcasts, no DRAM bounce).  ~9us/batch, fully overlapped
            # with the next batch's PE work.
            m1 = small.tile([1, 1], F32, tag="m1")
            nc.vector.reduce_max(m1, scores_sb, axis=AX.X)
            negm = small.tile([1, 1], F32, tag="negm")
            nc.vector.tensor_scalar_mul(negm, m1, -1.0)
            prob = sm1.tile([1, S], F32, tag="prob")
            nc.scalar.activation(prob, scores_sb, AF.Exp, bias=negm, scale=1.0)
            ssum = small.tile([1, 1], F32, tag="ssum")
            nc.vector.reduce_sum(ssum, prob, axis=AX.X)
            rtot = small.tile([1, 1], F32, tag="rtot")
            nc.vector.reciprocal(rtot, ssum)
            attn = sm1.tile([1, S], F32, tag="attn")
            nc.vector.tensor_scalar_mul(attn, prob, rtot)
            nc.sync.dma_start(out=out_d[bi], in_=attn)
            if debug_dumps:
                nc.sync.dma_start(out=dbg_scores[bi:bi + 1, :], in_=scores_sb)

        if debug_dumps:
            nc.sync.dma_start(out=dbg_hbt, in_=hbt_sb)
            nc.sync.dma_start(out=dbg_wet, in_=wet_sb)

    nc.compile()
    return nc


def _get_nc():
    if "nc" not in _CACHE:
        _CACHE["nc"] = _build()
    return _CACHE["nc"]


def kernel(hidden, encoder_outputs, W, b, v):
    from concourse.bass_utils import run_bass_kernel_spmd

    nc = _get_nc()
    hidden = np.ascontiguousarray(hidden, dtype=np.float32)
    encoder_outputs = np.ascontiguousarray(encoder_outputs, dtype=np.float32)
    W = np.ascontiguousarray(W, dtype=np.float32)
    b = np.ascontiguousarray(b, dtype=np.float32)
    v = np.ascontiguousarray(v, dtype=np.float32)

    in_maps = [
        {
            "hidden": hidden[c * BP:(c + 1) * BP],
            "enc": encoder_outputs[c * BP:(c + 1) * BP],
            "W": W,
            "b": b,
            "v": v,
        }
        for c in range(N_CORES)
    ]
    r = run_bass_kernel_spmd(nc, in_maps, list(range(N_CORES)))
    out = np.concatenate([r.results[c]["out"] for c in range(N_CORES)], axis=0)
    return out[:, None, :].astype(np.float32)

